# revision 22
# baseline (speedup 1.0000x reference)
"""Trainium2 Bass kernel for cross-attention (efficient/linear attention variant).

Computation per batch b (fully batch-independent -> data parallel over 8 cores):
    q  = Wq @ x[b]                         # (128, N)
    kv = Wkv @ context[b].T                # (256, NCTX)
    k, v = kv[:128], kv[128:]
    q = softmax_d(q) * d**-0.5             # softmax over feature dim within head
    k = softmax_n(k)                       # softmax over sequence dim
    ctx[h] = k_h @ v_h.T                   # (32, 32) per head
    out[h] = ctx[h].T @ q_h                # (32, N)
    y = Wo @ out + bo

Strategy (v3):
  - One batch per NeuronCore (8 cores), no collectives.
  - Host pre-transposes/tiles everything so all device DMAs are fully
    contiguous per partition; context streams as fp8-e3m4 (random per-n
    quantization noise averages out in the C/Z reduction), x/weights bf16.
    (fp16 context was tried: the fp16 LDWEIGHTS binds at ~119ns > the
    107ns N=256 matmul issue gap and costs +12us on the kv stream.)
  - Merged phase A+B1 loop over 32 tiles: kvT = ctxT_chunk.T @ WkvT (n on
    partitions; mixed fp8 x bf16 matmuls), exp(k) on ScalarE, one
    accumulating matmul per 128-chunk computes C[(d),(e)] = sum_n exp(k) v
    AND Z[d] = sum_n exp(k) via a ones column on v.  Concurrently (B1) the
    q-projection, exp(q), S = head-sums of exp(q) via a block-mask matmul,
    1/S via reciprocal_approx_fast (custom-DVE).
  - cz/sb run at LAG 2 (tile t does cz/sb for t-2, eqn for t-3): the
    ScalarE-exp -> DVE-cast -> PE chain gets a full extra tile of slack,
    removing the ~431ns/tile PE stall the lag-1 version had.
  - eqn = exp(q)/S is split GpSimd/DVE half-half so neither engine's
    FIFO falls behind the tile period (all-GpSimd lagged into B2).
  - Barrier: BD = blockdiag(C/Z); M = BD^T @ (Wo^T * scale) folded once on
    device, so phase B2 is only 2 matmuls/tile.
  - Phase B2: y = M^T @ eqn, PSUM evacuation split ScalarE/DVE halves,
    y_ps 4-deep (all 8 PSUM banks), yt staged as DOUBLE-tiles with one
    sync-ring DMA per 2 tiles (halves ring issue + completion receipts).
    y stays bf16: fp8 evacuation measured SLOWER (825 vs 690ns) and the
    batched DMA already removes the ring bottleneck.
    Junk work is avoided everywhere -- extra PE activity trips the P0
    power throttle (2.4 -> 2.0 GHz globally); the HAM pre-warm fills the
    otherwise-idle preamble window.
"""

import sys
from contextlib import ExitStack

import numpy as np

if "/opt/trn_rl_repo" not in sys.path:
    sys.path.insert(0, "/opt/trn_rl_repo")

import ml_dtypes

import concourse.bass as bass
from concourse import bacc
import concourse.mybir as mybir
import concourse.tile as tile
from concourse.bass_utils import run_bass_kernel_spmd

HEADS = 4
DIM_HEAD = 32
SCALE = DIM_HEAD**-0.5
B = 8
DIM = 256
N = 16384
NCTX = 16384
CDIM = 512
HID = HEADS * DIM_HEAD  # 128

BF16 = mybir.dt.bfloat16
F32 = mybir.dt.float32
FP8 = mybir.dt.float8e3
EXP = mybir.ActivationFunctionType.Exp

TILE_N = 512
NT = N // TILE_N  # 32 x-tiles (== context tiles)
NCHUNK = NCTX // 128  # 128 chunks in the C/Z accumulation


def build_graph(bias_zero: bool) -> bass.Bass:
    nc = bacc.Bacc()

    ctxt = nc.dram_tensor("ctxt", [NT, 128, 4 * TILE_N], FP8, kind="ExternalInput")
    xst = nc.dram_tensor("xst", [NT, 128, 2 * TILE_N], BF16, kind="ExternalInput")
    wqt = nc.dram_tensor("wqt", [128, 2, HID], BF16, kind="ExternalInput")
    wkvt = nc.dram_tensor("wkvt", [128, 4, 2 * HID], BF16, kind="ExternalInput")
    wot = nc.dram_tensor("wot", [HID, DIM], BF16, kind="ExternalInput")
    bob = nc.dram_tensor("bob", [128, 2], F32, kind="ExternalInput")
    bmask = nc.dram_tensor("bmask", [HID, HID], BF16, kind="ExternalInput")
    y = nc.dram_tensor("y", [NT, 128, 2 * TILE_N], BF16, kind="ExternalOutput")

    with tile.TileContext(nc) as tc, ExitStack() as ctx:
        cpool = ctx.enter_context(tc.tile_pool(name="consts", bufs=1))

        # wkvt first: needed by the very first matmul.  cc=0 alone first so
        # the first matmul's operands land as early as possible.
        wkvt_sb = cpool.tile([128, 4, 2 * HID], BF16)
        nc.sync.dma_start(wkvt_sb[:, 0:1, :], wkvt[:, 0:1, :])
        nc.sync.dma_start(wkvt_sb[:, 1:4, :], wkvt[:, 1:4, :])

        # persistent intermediates
        eq_all = cpool.tile([128, NT, TILE_N], BF16)  # exp(q), 32KB/part
        eqn_all = cpool.tile([128, NT, TILE_N], BF16)  # exp(q)/S, 32KB/part
        m_sb = cpool.tile([HID, DIM], BF16)  # folded BD^T @ WoT
        bd_sb = cpool.tile([HID, HID], BF16)
        bdt_sb = cpool.tile([HID, HID], BF16)

        # manual vt rotation: ones column at [:, :, HID] preset once
        vt_bufs = [cpool.tile([128, 4, 136], BF16, name=f"vtb{i}") for i in range(4)]
        for vtb in vt_bufs:
            nc.gpsimd.memset(vtb[:, :, HID : HID + 1], 1.0)

        czctx = ExitStack()
        czpool = czctx.enter_context(tc.tile_pool(name="czp", bufs=1, space="PSUM"))
        cz_ps = czpool.tile([128, HID + 1], F32)

        with (
            tc.tile_pool(name="actx", bufs=4) as apool,
            tc.tile_pool(name="axs", bufs=6) as xpool,
            tc.tile_pool(name="akv", bufs=4) as kpool,
            tc.tile_pool(name="aps", bufs=2, space="PSUM") as apsum,
        ):
            # context tile 0 (split in three for earlier compute start) + x0
            ct_tiles: dict = {}
            xs_tiles: dict = {}
            ct0 = apool.tile([128, 4 * TILE_N], FP8, tag="ct")
            ct0_r = ct0.rearrange("p (c n) -> p c n", c=4)
            src0 = ctxt[0].rearrange("p (c n) -> p c n", c=4)
            # first context tile issued on the scalar HWDGE ring, in parallel
            # with wkvt on the sync ring, so the first matmul starts earlier
            nc.scalar.dma_start(ct0_r[:, :, 0:128], src0[:, :, 0:128])
            nc.scalar.dma_start(ct0_r[:, :, 128:256], src0[:, :, 128:256])
            nc.scalar.dma_start(ct0_r[:, :, 256:TILE_N], src0[:, :, 256:TILE_N])
            xs0 = xpool.tile([128, 2 * TILE_N], BF16, tag="xt")
            nc.sync.dma_start(xs0, xst[0])
            ct_tiles[0] = ct0
            xs_tiles[0] = xs0

            wqt_sb = cpool.tile([128, 2, HID], BF16)
            nc.scalar.dma_start(wqt_sb, wqt[:, :, :])
            wot_sb = cpool.tile([HID, DIM], BF16)
            nc.sync.dma_start(wot_sb, wot[:, :])
            bo_sb = cpool.tile([128, 2], F32)
            nc.sync.dma_start(bo_sb, bob[:, :])
            bmask_sb = cpool.tile([HID, HID], BF16)
            nc.scalar.dma_start(bmask_sb, bmask[:, :])

            # HAM pre-warm: fill the PE-idle window between the framework
            # preamble (~7.2us) and first-DMA-landed (~10.6us) with junk
            # matmuls on uninitialized SBUF so the clock gate is already
            # open (K=8/8) when the real kv stream starts.  One-time ~3us
            # of PE activity -- no P0 power risk (unlike per-tile filler).
            warm_ps = apsum.tile([128, 4, 2 * HID], F32, tag="kvt")
            for w in range(8):
                nc.tensor.matmul(
                    warm_ps[:, 0:2, :].rearrange("p a b -> p (a b)"),
                    eq_all[:, 20, 0:HID],
                    eq_all[:, 21, :],
                    start=True,
                    stop=True,
                )

            # ------- merged Phase A + B1 (C/S work staggered TWO tiles) -----
            kts: dict = {}
            rss: dict = {}
            for t in range(NT + 3):
                if t < NT:
                    if t not in ct_tiles:
                        ct = apool.tile([128, 4 * TILE_N], FP8, tag="ct")
                        nc.sync.dma_start(ct, ctxt[t])
                        xs = xpool.tile([128, 2 * TILE_N], BF16, tag="xt")
                        nc.sync.dma_start(xs, xst[t])
                        ct_tiles[t] = ct
                        xs_tiles[t] = xs
                    ct = ct_tiles.pop(t)
                    xs = xs_tiles.pop(t)
                    # kv projection: 4 chunks of 128 n, contraction over 4 cc
                    kvt_ps = apsum.tile([128, 4, 2 * HID], F32, tag="kvt")
                    for j in range(4):
                        for cc in range(4):
                            nc.tensor.matmul(
                                kvt_ps[:, j, :],
                                ct[:, cc * TILE_N + j * 128 : cc * TILE_N + (j + 1) * 128],
                                wkvt_sb[:, cc, :],
                                start=(cc == 0),
                                stop=(cc == 3),
                            )
                    kt = kpool.tile([128, 4, HID], BF16, tag="kt")
                    nc.scalar.activation(kt, kvt_ps[:, :, 0:HID], EXP)
                    vt = vt_bufs[t % 4]
                    # vt copy on ScalarE too: single producer for the whole
                    # cz dependency chain (kt+vt), keeps the DVE FIFO out of
                    # the loop-critical path entirely
                    nc.scalar.copy(vt[:, :, 0:HID], kvt_ps[:, :, HID : 2 * HID])
                    kts[t] = kt
                if 2 <= t <= NT + 1:
                    u = t - 2
                    kt2 = kts.pop(u)
                    vt2 = vt_bufs[u % 4]
                    for j in range(4):
                        ci = u * 4 + j
                        nc.tensor.matmul(
                            cz_ps,
                            kt2[:, j, :],
                            vt2[:, j, 0 : HID + 1],
                            start=(ci == 0),
                            stop=(ci == NCHUNK - 1),
                        )
                    # B1: S broadcast to 128 rows via block mask, then 1/S
                    sb_ps = apsum.tile([128, TILE_N], F32, tag="sb", bufs=1)
                    nc.tensor.matmul(
                        sb_ps, bmask_sb, eq_all[:, u, :], start=True, stop=True
                    )
                    if u >= NT - 3:
                        # these rs outlive the merged-loop pools (their eqn
                        # is deferred past the barrier) -> persistent pool
                        rs = cpool.tile([128, TILE_N], F32, name=f"rslate{u}")
                    else:
                        rs = kpool.tile([128, TILE_N], F32, tag="rs")
                    nc.vector.reciprocal_approx_fast(rs, sb_ps)
                    rss[u] = rs
                if t < NT:
                    # B1: q projection + exp(q) (emitted AFTER cz/sb: the
                    # 2-tile-old cz deps are always ready, so the PE never
                    # idles waiting for this tile's xs DMA mid-block)
                    q_ps = apsum.tile([128, TILE_N], F32, tag="q")
                    for cc in range(2):
                        nc.tensor.matmul(
                            q_ps,
                            wqt_sb[:, cc, :],
                            xs[:, cc * TILE_N : (cc + 1) * TILE_N],
                            start=(cc == 0),
                            stop=(cc == 1),
                        )
                    nc.scalar.activation(eq_all[:, t, :], q_ps, EXP)
                if 3 <= t < NT:
                    u2 = t - 3
                    # DVE does recip + eqn only (~66% busy); vt moved to
                    # ScalarE and GpSimd left idle to cap peak concurrent
                    # engine power (P0 downclock risk)
                    nc.vector.tensor_mul(
                        eqn_all[:, u2, :], eq_all[:, u2, :], rss.pop(u2)
                    )
                # eqn for the last 3 tiles is deferred until after the
                # barrier DVE chain is emitted: keeps the DVE FIFO clear so
                # rz/bdf/bd/bdt (which gate ALL of B2 via m_sb) run first.

        # ------- barrier: M = blockdiag(C/Z)^T @ WoT ------------------------
        # Chain kept minimal -- it gates ALL of B2:
        #   cz -> [czT block-transpose (DVE) || rz (DVE)] -> 4 concurrent
        #   row+col-tiled 32-deep matmuls (mask-free: row tiling restricts
        #   the contraction to each head's diagonal block) -> m = m'' * 1/Z.
        # The bmask multiply is unnecessary because only the DIAGONAL
        # 32x32 blocks of C are read, and the DVE's per-block transpose is
        # exact on those.
        with tc.tile_pool(name="barp", bufs=1, space="PSUM") as barpsum:
            rz = cpool.tile([128, 1], F32)
            nc.vector.reciprocal(rz, cz_ps[:, HID : HID + 1])
            # cm = C * blockmask (bf16, straight from PSUM); the per-32x32-
            # block StreamTranspose IS the full transpose of the block-
            # diagonal cm; 1/Z folds into the final PSUM->SBUF copy, so the
            # chain is one DVE op shorter than computing BD=C/Z first.
            nc.vector.tensor_mul(bd_sb, cz_ps[:, 0:HID], bmask_sb)
            nc.vector.transpose(bdt_sb, bd_sb)
            m_ps = barpsum.tile([128, DIM], F32)
            nc.tensor.matmul(m_ps, bdt_sb, wot_sb, start=True, stop=True)
            nc.vector.tensor_scalar_mul(m_sb, m_ps, rz)
            # deferred eqn for the last 3 tiles (emitted AFTER the barrier
            # DVE ops so they don't block m_sb; B2 only reaches these tiles
            # ~20us later).  GpSimd takes them so the DVE queue stays clear
            # for B2 evacuation.
            for u2 in range(NT - 3, NT):
                rs = rss.pop(u2)
                nc.gpsimd.tensor_mul(eqn_all[:, u2, :], eq_all[:, u2, :], rs)
        czctx.close()

        # ------- Phase B2: y = M^T @ eqn, evacuation split ScalarE/DVE ------
        with (
            tc.tile_pool(name="bsb", bufs=2) as bpool,
            tc.tile_pool(name="bps", bufs=2, space="PSUM") as bpsum,
        ):
            yt = None
            for t in range(NT):
                y_ps = bpsum.tile([128, 2, TILE_N], F32, tag="y", bufs=4)
                for oc in range(2):
                    nc.tensor.matmul(
                        y_ps[:, oc, :],
                        m_sb[:, oc * HID : (oc + 1) * HID],
                        eqn_all[:, t, :],
                        start=True,
                        stop=True,
                    )
                # whole-tile evacuation alternating engines: a tile written
                # by TWO engines gets serialized by the scheduler (DVE half
                # was made to wait on the ScalarE half every tile); with one
                # writer per tile ScalarE and DVE each do every other tile
                # fully, in parallel (~685ns/tile effective each)
                if t % 2 == 0:
                    yt = bpool.tile([128, 2 * TILE_N], BF16, tag="yte", bufs=3)
                    if bias_zero:
                        nc.scalar.copy(yt, y_ps.rearrange("p a b -> p (a b)"))
                    else:
                        nc.scalar.add(yt[:, 0:TILE_N], y_ps[:, 0, :], bo_sb[:, 0:1])
                        nc.scalar.add(yt[:, TILE_N:], y_ps[:, 1, :], bo_sb[:, 1:2])
                else:
                    yt = bpool.tile([128, 2 * TILE_N], BF16, tag="yto", bufs=3)
                    if bias_zero:
                        nc.vector.tensor_copy(yt, y_ps.rearrange("p a b -> p (a b)"))
                    else:
                        nc.vector.tensor_scalar_add(
                            yt[:, 0:TILE_N], y_ps[:, 0, :], bo_sb[:, 0:1]
                        )
                        nc.vector.tensor_scalar_add(
                            yt[:, TILE_N:], y_ps[:, 1, :], bo_sb[:, 1:2]
                        )
                nc.sync.dma_start(y[t], yt)

    nc.compile()
    return nc


_GRAPH_CACHE: dict = {}


def _prep_inputs(x, context, Wq, Wkv, Wo, bo):
    bf16 = ml_dtypes.bfloat16
    x = np.asarray(x, dtype=np.float32)
    context = np.asarray(context, dtype=np.float32)
    Wq = np.asarray(Wq, dtype=np.float32)
    Wkv = np.asarray(Wkv, dtype=np.float32)
    Wo = np.asarray(Wo, dtype=np.float32)
    bo = np.asarray(bo, dtype=np.float32)

    # [128, 2, HID]: wqt[p, cc, m] = Wq[m, cc*128+p]
    wqt = np.ascontiguousarray(Wq.T.reshape(2, 128, HID).transpose(1, 0, 2)).astype(bf16)
    # [128, 4, 256]: wkvt[p, cc, o] = Wkv[o, cc*128+p]
    wkvt = np.ascontiguousarray(Wkv.T.reshape(4, 128, 2 * HID).transpose(1, 0, 2)).astype(bf16)
    # [HID, DIM]: wot[e, o] = Wo[o, e] * SCALE
    wot = np.ascontiguousarray((Wo * SCALE).T).astype(bf16)
    bob = np.ascontiguousarray(bo.reshape(2, 128).T).astype(np.float32)

    bmask = (
        (np.arange(HID)[:, None] // DIM_HEAD) == (np.arange(HID)[None, :] // DIM_HEAD)
    ).astype(bf16)

    in_maps = []
    for b in range(B):
        # ctxt[t, p, cc*512+j] = context[b, t*512+j, cc*128+p]
        ctx_t = np.ascontiguousarray(
            context[b].reshape(NT, TILE_N, 4, 128).transpose(0, 3, 2, 1).reshape(NT, 128, 4 * TILE_N)
        ).astype(ml_dtypes.float8_e3m4)
        # xst[t, p, cc*512+j] = x[b, cc*128+p, t*512+j]
        xs_t = np.ascontiguousarray(
            x[b].reshape(2, 128, NT, TILE_N).transpose(2, 1, 0, 3).reshape(NT, 128, 2 * TILE_N)
        ).astype(bf16)
        in_maps.append(
            {
                "ctxt": ctx_t,
                "xst": xs_t,
                "wqt": wqt,
                "wkvt": wkvt,
                "wot": wot,
                "bob": bob,
                "bmask": bmask,
            }
        )
    return in_maps


def run(inputs: dict, trace: bool = False):
    bias_zero = bool(np.all(np.asarray(inputs["bo"]) == 0))
    key = ("nc", bias_zero)
    if key not in _GRAPH_CACHE:
        _GRAPH_CACHE[key] = build_graph(bias_zero)
    nc = _GRAPH_CACHE[key]
    in_maps = _prep_inputs(**inputs)
    res = run_bass_kernel_spmd(nc, in_maps, core_ids=list(range(B)), trace=trace)
    out = np.stack(
        [
            np.asarray(res.results[b]["y"], dtype=np.float32)
            .reshape(NT, 128, 2, TILE_N)
            .transpose(2, 1, 0, 3)
            .reshape(DIM, N)
            for b in range(B)
        ]
    )
    return out, res


def kernel(**inputs) -> np.ndarray:
    out, _ = run(inputs, trace=False)
    return out


# revision 23
# speedup vs baseline: 1.0547x; 1.0547x over previous
"""Trainium2 Bass kernel for cross-attention (efficient/linear attention variant).

Computation per batch b (fully batch-independent -> data parallel over 8 cores):
    q  = Wq @ x[b]                         # (128, N)
    kv = Wkv @ context[b].T                # (256, NCTX)
    k, v = kv[:128], kv[128:]
    q = softmax_d(q) * d**-0.5             # softmax over feature dim within head
    k = softmax_n(k)                       # softmax over sequence dim
    ctx[h] = k_h @ v_h.T                   # (32, 32) per head
    out[h] = ctx[h].T @ q_h                # (32, N)
    y = Wo @ out + bo

Strategy (v3):
  - One batch per NeuronCore (8 cores), no collectives.
  - Host pre-transposes/tiles everything so all device DMAs are fully
    contiguous per partition; context streams as fp8-e3m4 (random per-n
    quantization noise averages out in the C/Z reduction), x/weights bf16.
    (fp16 context was tried: the fp16 LDWEIGHTS binds at ~119ns > the
    107ns N=256 matmul issue gap and costs +12us on the kv stream.)
  - Merged phase A+B1 loop over 32 tiles: kvT = ctxT_chunk.T @ WkvT (n on
    partitions; mixed fp8 x bf16 matmuls), exp(k) on ScalarE, one
    accumulating matmul per 128-chunk computes C[(d),(e)] = sum_n exp(k) v
    AND Z[d] = sum_n exp(k) via a ones column on v.  Concurrently (B1) the
    q-projection, exp(q), S = head-sums of exp(q) via a block-mask matmul,
    1/S via reciprocal_approx_fast (custom-DVE).
  - cz/sb run at LAG 2 (tile t does cz/sb for t-2, eqn for t-3): the
    ScalarE-exp -> DVE-cast -> PE chain gets a full extra tile of slack,
    removing the ~431ns/tile PE stall the lag-1 version had.
  - eqn = exp(q)/S is split GpSimd/DVE half-half so neither engine's
    FIFO falls behind the tile period (all-GpSimd lagged into B2).
  - Barrier: BD = blockdiag(C/Z); M = BD^T @ (Wo^T * scale) folded once on
    device, so phase B2 is only 2 matmuls/tile.
  - Phase B2: y = M^T @ eqn, PSUM evacuation split ScalarE/DVE halves,
    y_ps 4-deep (all 8 PSUM banks), yt staged as DOUBLE-tiles with one
    sync-ring DMA per 2 tiles (halves ring issue + completion receipts).
    y stays bf16: fp8 evacuation measured SLOWER (825 vs 690ns) and the
    batched DMA already removes the ring bottleneck.
    Junk work is avoided everywhere -- extra PE activity trips the P0
    power throttle (2.4 -> 2.0 GHz globally); the HAM pre-warm fills the
    otherwise-idle preamble window.
"""

import sys
from contextlib import ExitStack

import numpy as np

if "/opt/trn_rl_repo" not in sys.path:
    sys.path.insert(0, "/opt/trn_rl_repo")

import ml_dtypes

import concourse.bass as bass
from concourse import bacc
import concourse.mybir as mybir
import concourse.tile as tile
from concourse.bass_utils import run_bass_kernel_spmd

HEADS = 4
DIM_HEAD = 32
SCALE = DIM_HEAD**-0.5
B = 8
DIM = 256
N = 16384
NCTX = 16384
CDIM = 512
HID = HEADS * DIM_HEAD  # 128

BF16 = mybir.dt.bfloat16
F32 = mybir.dt.float32
FP8 = mybir.dt.float8e3
EXP = mybir.ActivationFunctionType.Exp

TILE_N = 512
NT = N // TILE_N  # 32 x-tiles (== context tiles)
NCHUNK = NCTX // 128  # 128 chunks in the C/Z accumulation


def build_graph(bias_zero: bool) -> bass.Bass:
    nc = bacc.Bacc()

    ctxt = nc.dram_tensor("ctxt", [NT, 128, 4 * TILE_N], FP8, kind="ExternalInput")
    xst = nc.dram_tensor("xst", [NT, 128, 2 * TILE_N], BF16, kind="ExternalInput")
    wqt = nc.dram_tensor("wqt", [128, 2, HID], BF16, kind="ExternalInput")
    wkvt = nc.dram_tensor("wkvt", [128, 4, 2 * HID], BF16, kind="ExternalInput")
    wot = nc.dram_tensor("wot", [HID, DIM], BF16, kind="ExternalInput")
    bob = nc.dram_tensor("bob", [128, 2], F32, kind="ExternalInput")
    bmask = nc.dram_tensor("bmask", [HID, HID], BF16, kind="ExternalInput")
    y = nc.dram_tensor("y", [NT, 128, 2 * TILE_N], BF16, kind="ExternalOutput")

    with tile.TileContext(nc) as tc, ExitStack() as ctx:
        cpool = ctx.enter_context(tc.tile_pool(name="consts", bufs=1))

        # wkvt first: needed by the very first matmul.  cc=0 alone first so
        # the first matmul's operands land as early as possible.
        wkvt_sb = cpool.tile([128, 4, 2 * HID], BF16)
        nc.sync.dma_start(wkvt_sb[:, 0:1, :], wkvt[:, 0:1, :])
        nc.sync.dma_start(wkvt_sb[:, 1:4, :], wkvt[:, 1:4, :])

        # persistent intermediates
        eq_all = cpool.tile([128, NT, TILE_N], BF16)  # exp(q), 32KB/part
        eqn_all = cpool.tile([128, NT, TILE_N], BF16)  # exp(q)/S, 32KB/part
        m_sb = cpool.tile([HID, DIM], BF16)  # folded BD^T @ WoT
        bd_sb = cpool.tile([HID, HID], BF16)
        bdt_sb = cpool.tile([HID, HID], BF16)

        # manual vt rotation: ones column at [:, :, HID] preset once
        vt_bufs = [cpool.tile([128, 4, 136], BF16, name=f"vtb{i}") for i in range(4)]
        for vtb in vt_bufs:
            nc.gpsimd.memset(vtb[:, :, HID : HID + 1], 1.0)

        czctx = ExitStack()
        czpool = czctx.enter_context(tc.tile_pool(name="czp", bufs=1, space="PSUM"))
        cz_ps = czpool.tile([128, HID + 1], F32)

        with (
            tc.tile_pool(name="actx", bufs=4) as apool,
            tc.tile_pool(name="axs", bufs=6) as xpool,
            tc.tile_pool(name="akv", bufs=4) as kpool,
            tc.tile_pool(name="aps", bufs=2, space="PSUM") as apsum,
        ):
            # context tile 0 (split in three for earlier compute start) + x0
            ct_tiles: dict = {}
            xs_tiles: dict = {}
            ct0 = apool.tile([128, 4 * TILE_N], FP8, tag="ct")
            ct0_r = ct0.rearrange("p (c n) -> p c n", c=4)
            src0 = ctxt[0].rearrange("p (c n) -> p c n", c=4)
            # first context tile issued on the scalar HWDGE ring, in parallel
            # with wkvt on the sync ring, so the first matmul starts earlier
            nc.scalar.dma_start(ct0_r[:, :, 0:128], src0[:, :, 0:128])
            nc.scalar.dma_start(ct0_r[:, :, 128:256], src0[:, :, 128:256])
            nc.scalar.dma_start(ct0_r[:, :, 256:TILE_N], src0[:, :, 256:TILE_N])
            xs0 = xpool.tile([128, 2 * TILE_N], BF16, tag="xt")
            nc.sync.dma_start(xs0, xst[0])
            ct_tiles[0] = ct0
            xs_tiles[0] = xs0

            wqt_sb = cpool.tile([128, 2, HID], BF16)
            nc.scalar.dma_start(wqt_sb, wqt[:, :, :])
            wot_sb = cpool.tile([HID, DIM], BF16)
            nc.sync.dma_start(wot_sb, wot[:, :])
            bo_sb = cpool.tile([128, 2], F32)
            nc.sync.dma_start(bo_sb, bob[:, :])
            bmask_sb = cpool.tile([HID, HID], BF16)
            nc.scalar.dma_start(bmask_sb, bmask[:, :])

            # HAM pre-warm: fill the PE-idle window between the framework
            # preamble (~7.2us) and first-DMA-landed (~10.6us) with junk
            # matmuls on uninitialized SBUF so the clock gate is already
            # open (K=8/8) when the real kv stream starts.  One-time ~3us
            # of PE activity -- no P0 power risk (unlike per-tile filler).
            warm_ps = apsum.tile([128, 4, 2 * HID], F32, tag="kvt")
            for w in range(8):
                nc.tensor.matmul(
                    warm_ps[:, 0:2, :].rearrange("p a b -> p (a b)"),
                    eq_all[:, 20, 0:HID],
                    eq_all[:, 21, :],
                    start=True,
                    stop=True,
                )

            # ------- merged Phase A + B1 (C/S work staggered TWO tiles) -----
            kts: dict = {}
            rss: dict = {}
            for t in range(NT + 3):
                if t < NT:
                    if t not in ct_tiles:
                        ct = apool.tile([128, 4 * TILE_N], FP8, tag="ct")
                        nc.sync.dma_start(ct, ctxt[t])
                        xs = xpool.tile([128, 2 * TILE_N], BF16, tag="xt")
                        nc.sync.dma_start(xs, xst[t])
                        ct_tiles[t] = ct
                        xs_tiles[t] = xs
                    ct = ct_tiles.pop(t)
                    xs = xs_tiles.pop(t)
                    # kv projection: 4 chunks of 128 n, contraction over 4 cc
                    kvt_ps = apsum.tile([128, 4, 2 * HID], F32, tag="kvt")
                    for j in range(4):
                        for cc in range(4):
                            nc.tensor.matmul(
                                kvt_ps[:, j, :],
                                ct[:, cc * TILE_N + j * 128 : cc * TILE_N + (j + 1) * 128],
                                wkvt_sb[:, cc, :],
                                start=(cc == 0),
                                stop=(cc == 3),
                            )
                    kt = kpool.tile([128, 4, HID], BF16, tag="kt")
                    nc.scalar.activation(kt, kvt_ps[:, :, 0:HID], EXP)
                    vt = vt_bufs[t % 4]
                    # vt copy on ScalarE too: single producer for the whole
                    # cz dependency chain (kt+vt), keeps the DVE FIFO out of
                    # the loop-critical path entirely
                    nc.scalar.copy(vt[:, :, 0:HID], kvt_ps[:, :, HID : 2 * HID])
                    kts[t] = kt
                    # B1: q projection + exp(q)
                    q_ps = apsum.tile([128, TILE_N], F32, tag="q")
                    for cc in range(2):
                        nc.tensor.matmul(
                            q_ps,
                            wqt_sb[:, cc, :],
                            xs[:, cc * TILE_N : (cc + 1) * TILE_N],
                            start=(cc == 0),
                            stop=(cc == 1),
                        )
                    nc.scalar.activation(eq_all[:, t, :], q_ps, EXP)
                if 2 <= t <= NT + 1:
                    u = t - 2
                    kt2 = kts.pop(u)
                    vt2 = vt_bufs[u % 4]
                    for j in range(4):
                        ci = u * 4 + j
                        nc.tensor.matmul(
                            cz_ps,
                            kt2[:, j, :],
                            vt2[:, j, 0 : HID + 1],
                            start=(ci == 0),
                            stop=(ci == NCHUNK - 1),
                        )
                    # B1: S broadcast to 128 rows via block mask, then 1/S
                    sb_ps = apsum.tile([128, TILE_N], F32, tag="sb", bufs=1)
                    nc.tensor.matmul(
                        sb_ps, bmask_sb, eq_all[:, u, :], start=True, stop=True
                    )
                    if u >= NT - 3:
                        # these rs outlive the merged-loop pools (their eqn
                        # is deferred past the barrier) -> persistent pool
                        rs = cpool.tile([128, TILE_N], F32, name=f"rslate{u}")
                    else:
                        rs = kpool.tile([128, TILE_N], F32, tag="rs")
                    nc.vector.reciprocal_approx_fast(rs, sb_ps)
                    rss[u] = rs
                if 3 <= t < NT:
                    u2 = t - 3
                    # DVE does recip + eqn only (~66% busy); vt moved to
                    # ScalarE and GpSimd left idle to cap peak concurrent
                    # engine power (P0 downclock risk)
                    nc.vector.tensor_mul(
                        eqn_all[:, u2, :], eq_all[:, u2, :], rss.pop(u2)
                    )
                # eqn for the last 3 tiles is deferred until after the
                # barrier DVE chain is emitted: keeps the DVE FIFO clear so
                # rz/bdf/bd/bdt (which gate ALL of B2 via m_sb) run first.

        # ------- barrier: M = blockdiag(C/Z)^T @ WoT ------------------------
        # Chain kept minimal -- it gates ALL of B2:
        #   cz -> [czT block-transpose (DVE) || rz (DVE)] -> 4 concurrent
        #   row+col-tiled 32-deep matmuls (mask-free: row tiling restricts
        #   the contraction to each head's diagonal block) -> m = m'' * 1/Z.
        # The bmask multiply is unnecessary because only the DIAGONAL
        # 32x32 blocks of C are read, and the DVE's per-block transpose is
        # exact on those.
        with tc.tile_pool(name="barp", bufs=1, space="PSUM") as barpsum:
            rz = cpool.tile([128, 1], F32)
            nc.vector.reciprocal(rz, cz_ps[:, HID : HID + 1])
            # cm = C * blockmask (bf16, straight from PSUM); the per-32x32-
            # block StreamTranspose IS the full transpose of the block-
            # diagonal cm; 1/Z folds into the final PSUM->SBUF copy, so the
            # chain is one DVE op shorter than computing BD=C/Z first.
            nc.vector.tensor_mul(bd_sb, cz_ps[:, 0:HID], bmask_sb)
            nc.vector.transpose(bdt_sb, bd_sb)
            m_ps = barpsum.tile([128, DIM], F32)
            nc.tensor.matmul(m_ps, bdt_sb, wot_sb, start=True, stop=True)
            nc.vector.tensor_scalar_mul(m_sb, m_ps, rz)
            # deferred eqn for the last 3 tiles (emitted AFTER the barrier
            # DVE ops so they don't block m_sb; B2 only reaches these tiles
            # ~20us later).  GpSimd takes them so the DVE queue stays clear
            # for B2 evacuation.
            for u2 in range(NT - 3, NT):
                rs = rss.pop(u2)
                nc.gpsimd.tensor_mul(eqn_all[:, u2, :], eq_all[:, u2, :], rs)
        czctx.close()

        # ------- Phase B2: y = M^T @ eqn, evacuation split ScalarE/DVE ------
        with (
            tc.tile_pool(name="bsb", bufs=2) as bpool,
            tc.tile_pool(name="bps", bufs=2, space="PSUM") as bpsum,
        ):
            yt = None
            for t in range(NT):
                y_ps = bpsum.tile([128, 2, TILE_N], F32, tag="y", bufs=4)
                for oc in range(2):
                    nc.tensor.matmul(
                        y_ps[:, oc, :],
                        m_sb[:, oc * HID : (oc + 1) * HID],
                        eqn_all[:, t, :],
                        start=True,
                        stop=True,
                    )
                # whole-tile evacuation alternating engines: a tile written
                # by TWO engines gets serialized by the scheduler (DVE half
                # was made to wait on the ScalarE half every tile); with one
                # writer per tile ScalarE and DVE each do every other tile
                # fully, in parallel (~685ns/tile effective each)
                if t % 2 == 0:
                    yt = bpool.tile([128, 2 * TILE_N], BF16, tag="yte", bufs=3)
                    if bias_zero:
                        nc.scalar.copy(yt, y_ps.rearrange("p a b -> p (a b)"))
                    else:
                        nc.scalar.add(yt[:, 0:TILE_N], y_ps[:, 0, :], bo_sb[:, 0:1])
                        nc.scalar.add(yt[:, TILE_N:], y_ps[:, 1, :], bo_sb[:, 1:2])
                else:
                    yt = bpool.tile([128, 2 * TILE_N], BF16, tag="yto", bufs=3)
                    if bias_zero:
                        nc.vector.tensor_copy(yt, y_ps.rearrange("p a b -> p (a b)"))
                    else:
                        nc.vector.tensor_scalar_add(
                            yt[:, 0:TILE_N], y_ps[:, 0, :], bo_sb[:, 0:1]
                        )
                        nc.vector.tensor_scalar_add(
                            yt[:, TILE_N:], y_ps[:, 1, :], bo_sb[:, 1:2]
                        )
                nc.sync.dma_start(y[t], yt)

    nc.compile()
    return nc


_GRAPH_CACHE: dict = {}


def _prep_inputs(x, context, Wq, Wkv, Wo, bo):
    bf16 = ml_dtypes.bfloat16
    x = np.asarray(x, dtype=np.float32)
    context = np.asarray(context, dtype=np.float32)
    Wq = np.asarray(Wq, dtype=np.float32)
    Wkv = np.asarray(Wkv, dtype=np.float32)
    Wo = np.asarray(Wo, dtype=np.float32)
    bo = np.asarray(bo, dtype=np.float32)

    # [128, 2, HID]: wqt[p, cc, m] = Wq[m, cc*128+p]
    wqt = np.ascontiguousarray(Wq.T.reshape(2, 128, HID).transpose(1, 0, 2)).astype(bf16)
    # [128, 4, 256]: wkvt[p, cc, o] = Wkv[o, cc*128+p]
    wkvt = np.ascontiguousarray(Wkv.T.reshape(4, 128, 2 * HID).transpose(1, 0, 2)).astype(bf16)
    # [HID, DIM]: wot[e, o] = Wo[o, e] * SCALE
    wot = np.ascontiguousarray((Wo * SCALE).T).astype(bf16)
    bob = np.ascontiguousarray(bo.reshape(2, 128).T).astype(np.float32)

    bmask = (
        (np.arange(HID)[:, None] // DIM_HEAD) == (np.arange(HID)[None, :] // DIM_HEAD)
    ).astype(bf16)

    in_maps = []
    for b in range(B):
        # ctxt[t, p, cc*512+j] = context[b, t*512+j, cc*128+p]
        ctx_t = np.ascontiguousarray(
            context[b].reshape(NT, TILE_N, 4, 128).transpose(0, 3, 2, 1).reshape(NT, 128, 4 * TILE_N)
        ).astype(ml_dtypes.float8_e3m4)
        # xst[t, p, cc*512+j] = x[b, cc*128+p, t*512+j]
        xs_t = np.ascontiguousarray(
            x[b].reshape(2, 128, NT, TILE_N).transpose(2, 1, 0, 3).reshape(NT, 128, 2 * TILE_N)
        ).astype(bf16)
        in_maps.append(
            {
                "ctxt": ctx_t,
                "xst": xs_t,
                "wqt": wqt,
                "wkvt": wkvt,
                "wot": wot,
                "bob": bob,
                "bmask": bmask,
            }
        )
    return in_maps


def run(inputs: dict, trace: bool = False):
    bias_zero = bool(np.all(np.asarray(inputs["bo"]) == 0))
    key = ("nc", bias_zero)
    if key not in _GRAPH_CACHE:
        _GRAPH_CACHE[key] = build_graph(bias_zero)
    nc = _GRAPH_CACHE[key]
    in_maps = _prep_inputs(**inputs)
    res = run_bass_kernel_spmd(nc, in_maps, core_ids=list(range(B)), trace=trace)
    out = np.stack(
        [
            np.asarray(res.results[b]["y"], dtype=np.float32)
            .reshape(NT, 128, 2, TILE_N)
            .transpose(2, 1, 0, 3)
            .reshape(DIM, N)
            for b in range(B)
        ]
    )
    return out, res


def kernel(**inputs) -> np.ndarray:
    out, _ = run(inputs, trace=False)
    return out
